# revision 8
# baseline (speedup 1.0000x reference)
"""Trainium2 Bass kernel for a dense transformer block (B=8, N=1024, C=768, H=12).

Sharding: data-parallel over batch -- one batch element per NeuronCore (8 cores),
weights replicated, no collectives.

Per-core dataflow (x_b: [1024, 768]):
  LN1 (token-major, gain/bias folded into qkv_w on host)
  -> transpose to feature-major hT [768, 1024]
  -> QKV: q,k feature-major [64, 1024]/head; v token-major (+ ones column)
  -> per head: scores^T [keys, queries] = kT.T @ qT (scale folded into Wq),
     + rel-bias Toeplitz band (host-precomputed, bf16), exp (no max-sub; scores
     are small by construction), AV matmul with ones column -> unnormalized
     attn out (feature-major) + softmax sums; normalize via reciprocal +
     PE broadcast + multiply
  -> proj (feature-major) -> transpose -> +x residual (in-place) -> LN2
  -> transpose -> MLP fc1+gelu / fc2 (feature-major) -> transpose -> +residual.
"""

import os

import numpy as np

B, N, C, H, D = 8, 1024, 768, 12, 64
NT = N // 128   # 8 token tiles
KT = C // 128   # 6 feature tiles
F1 = 4 * C      # 3072
RT = F1 // 128  # 24
W = 2 * N - 1   # 2047 toeplitz band width
EPS = 1e-5

LAST_RESULTS = None  # stash of the last BassKernelResults (for test.py)

_NC_CACHE = {}


def _build_nc():
    from contextlib import ExitStack

    import concourse.bacc as bacc
    import concourse.tile as tile
    from concourse import masks, mybir

    f32 = mybir.dt.float32
    bf16 = mybir.dt.bfloat16
    AF = mybir.ActivationFunctionType
    AX = mybir.AxisListType
    OP = mybir.AluOpType

    nc = bacc.Bacc(
        "TRN2",
        target_bir_lowering=False,
        debug=False,
        enable_asserts=False,
        num_devices=8,
    )

    x_d = nc.dram_tensor("x", [N, C], f32, kind="ExternalInput").ap()
    wqkv_d = nc.dram_tensor("wqkvT", [C, 3 * C], f32, kind="ExternalInput").ap()
    bqkv_d = nc.dram_tensor("bqkv", [1, 3 * C], f32, kind="ExternalInput").ap()
    wproj_d = nc.dram_tensor("wprojT", [C, C], f32, kind="ExternalInput").ap()
    bproj_d = nc.dram_tensor("bproj", [1, C], f32, kind="ExternalInput").ap()
    wfc1_d = nc.dram_tensor("wfc1t", [RT, 128, C], f32, kind="ExternalInput").ap()
    bfc1_d = nc.dram_tensor("bfc1", [1, F1], f32, kind="ExternalInput").ap()
    wfc2_d = nc.dram_tensor("wfc2T", [F1, C], f32, kind="ExternalInput").ap()
    bfc2_d = nc.dram_tensor("bfc2", [1, C], f32, kind="ExternalInput").ap()
    rb_d = nc.dram_tensor("rband", [H, 128, W], bf16, kind="ExternalInput").ap()
    out_d = nc.dram_tensor("out", [N, C], f32, kind="ExternalOutput").ap()

    with tile.TileContext(nc) as tc, ExitStack() as ctx:
        # ---------------- kernel-wide pools (opened first, closed last: LIFO ok)
        cpool = ctx.enter_context(tc.tile_pool(name="const", bufs=1))
        ident = cpool.tile([128, 128], f32, tag="ident")
        masks.make_identity(nc, ident[:])
        onesP = cpool.tile([1, 128], f32, tag="onesP")
        nc.any.memset(onesP[:], 1.0)
        ones64 = cpool.tile([1, 64], f32, tag="ones64")
        nc.any.memset(ones64[:], 1.0)
        epsc = cpool.tile([128, 1], f32, tag="eps")
        nc.any.memset(epsc[:], EPS)
        bqkv_sb = cpool.tile([128, 18], f32, tag="bqkv")
        nc.sync.dma_start(bqkv_sb[:], bqkv_d[0].rearrange("(a p) -> p a", p=128))
        bv_row = cpool.tile([1, C], f32, tag="bvrow")
        nc.sync.dma_start(bv_row[:], bqkv_d[:, 2 * C :])
        bproj_sb = cpool.tile([128, 6], f32, tag="bproj")
        nc.sync.dma_start(bproj_sb[:], bproj_d[0].rearrange("(a p) -> p a", p=128))
        bfc1_sb = cpool.tile([128, RT], f32, tag="bfc1")
        nc.sync.dma_start(bfc1_sb[:], bfc1_d[0].rearrange("(a p) -> p a", p=128))
        bfc2_sb = cpool.tile([128, 6], f32, tag="bfc2")
        nc.sync.dma_start(bfc2_sb[:], bfc2_d[0].rearrange("(a p) -> p a", p=128))

        stat = ctx.enter_context(tc.tile_pool(name="stat", bufs=4))
        # chain pool: big buffers with slot-cycling via shared tags
        chain = ctx.enter_context(tc.tile_pool(name="chain", bufs=1))

        def fm_tile(name):
            return chain.tile([128, N], f32, tag="fm1024", bufs=18, name=name)

        def layernorm(dst_ap, src_ap, scratch_ap):
            """dst = (src - mean(src)) * rsqrt(var(src) + eps); scratch may alias dst."""
            sums = stat.tile([128, 1], f32, tag="sums", name="sums")
            nc.vector.reduce_sum(sums[:], src_ap, axis=AX.X)
            mu = stat.tile([128, 1], f32, tag="mu", name="mu")
            nc.vector.tensor_scalar_mul(mu[:], sums[:], 1.0 / C)
            nc.vector.tensor_mul(scratch_ap, src_ap, src_ap)
            ssq = stat.tile([128, 1], f32, tag="ssq", name="ssq")
            nc.vector.reduce_sum(ssq[:], scratch_ap, axis=AX.X)
            musq = stat.tile([128, 1], f32, tag="musq", name="musq")
            nc.vector.tensor_mul(musq[:], mu[:], mu[:])
            var = stat.tile([128, 1], f32, tag="var", name="var")
            nc.vector.tensor_scalar(
                var[:], ssq[:], 1.0 / C, musq[:], op0=OP.mult, op1=OP.subtract
            )
            sd = stat.tile([128, 1], f32, tag="sd", name="sd")
            nc.scalar.activation(sd[:], var[:], AF.Sqrt, bias=epsc[:])
            rstd = stat.tile([128, 1], f32, tag="rstd", name="rstd")
            nc.vector.reciprocal(rstd[:], sd[:])
            nmr = stat.tile([128, 1], f32, tag="nmr", name="nmr")
            nc.vector.tensor_scalar(
                nmr[:], mu[:], rstd[:], -1.0, op0=OP.mult, op1=OP.mult
            )
            nc.scalar.activation(dst_ap, src_ap, AF.Identity, bias=nmr[:], scale=rstd[:])

        # persistent per-batch state
        xs = [chain.tile([128, C], f32, tag="x", bufs=NT, name=f"x{t}") for t in range(NT)]
        hT = [fm_tile(f"hT{i}") for i in range(KT)]
        vaug = [
            chain.tile([128, H * 65], bf16, tag="vaug", bufs=NT, name=f"vaug{t}")
            for t in range(NT)
        ]
        s_all = chain.tile([H, N], f32, tag="sall", bufs=1, name="sall")

        # ---------------- phase A+B: load x, LN1, transpose -> hT
        with tc.tile_pool(name="psB", bufs=4, space="PSUM") as psB:
            for t in range(NT):
                nc.sync.dma_start(xs[t][:], x_d[t * 128 : (t + 1) * 128, :])
                h1 = chain.tile([128, C], f32, tag="hln", bufs=3, name=f"h1_{t}")
                layernorm(h1[:], xs[t][:], h1[:])
                for ct in range(KT):
                    ps = psB.tile([128, 128], f32, tag="tp", name="psb")
                    nc.tensor.transpose(
                        ps[:], h1[:, ct * 128 : (ct + 1) * 128], ident[:]
                    )
                    nc.any.tensor_copy(hT[ct][:, t * 128 : (t + 1) * 128], ps[:])

        # ---------------- phase C: QKV
        qkT = [fm_tile(f"qkT{i}") for i in range(12)]
        with tc.tile_pool(name="wqkv", bufs=KT) as wq_pool:
            wq = []
            for ct in range(KT):
                wt = wq_pool.tile([128, 3 * C], f32, tag="wq", name=f"wq{ct}")
                nc.sync.dma_start(wt[:], wqkv_d[ct * 128 : (ct + 1) * 128, :])
                wq.append(wt)
            with tc.tile_pool(name="psC", bufs=3, space="PSUM") as psC:
                # q,k feature-major
                for jt in range(12):
                    for qc in range(2):
                        ps = psC.tile([128, 512], f32, tag="ps", name="psc")
                        for ct in range(KT):
                            nc.tensor.matmul(
                                ps[:],
                                wq[ct][:, jt * 128 : (jt + 1) * 128],
                                hT[ct][:, qc * 512 : (qc + 1) * 512],
                                start=(ct == 0),
                                stop=(ct == KT - 1),
                            )
                        nc.scalar.activation(
                            qkT[jt][:, qc * 512 : (qc + 1) * 512],
                            ps[:],
                            AF.Identity,
                            bias=bqkv_sb[:, jt : jt + 1],
                        )
                # v token-major, bias via rank-1 ones matmul, ones col for sums
                for t in range(NT):
                    vview = vaug[t][:].rearrange("p (h e) -> p h e", e=65)
                    for vc in range(2):
                        ps = psC.tile([128, 384], f32, tag="psv", bufs=2, name="psv")
                        for ct in range(KT):
                            nc.tensor.matmul(
                                ps[:],
                                hT[ct][:, t * 128 : (t + 1) * 128],
                                wq[ct][:, 2 * C + vc * 384 : 2 * C + (vc + 1) * 384],
                                start=(ct == 0),
                                stop=False,
                            )
                        nc.tensor.matmul(
                            ps[:],
                            onesP[:],
                            bv_row[:, vc * 384 : (vc + 1) * 384],
                            start=False,
                            stop=True,
                        )
                        nc.vector.tensor_copy(
                            vview[:, vc * 6 : (vc + 1) * 6, 0:64],
                            ps[:].rearrange("p (h e) -> p h e", e=64),
                        )
                    nc.any.memset(vview[:, :, 64:65], 1.0)

        # ---------------- phase D: attention
        aT = [fm_tile(f"aT{i}") for i in range(KT)]
        with (
            tc.tile_pool(name="rbp", bufs=3) as rbp,
            tc.tile_pool(name="ptp", bufs=16) as ptp,
            tc.tile_pool(name="srowp", bufs=4) as srowp,
            tc.tile_pool(name="oddp", bufs=3) as oddp,
            tc.tile_pool(name="psS", bufs=2, space="PSUM") as psS,
            tc.tile_pool(name="psAV", bufs=3, space="PSUM") as psAV,
        ):
            for h in range(H):
                hp = h // 2
                odd = h % 2
                ro = odd * 64
                rb = rbp.tile([128, W], bf16, tag="rb", name=f"rb{h}")
                nc.sync.dma_start(rb[:], rb_d[h])
                ptiles = []
                for kc in range(NT):
                    ps = psS.tile([128, 1024], f32, tag="ps", name="pss")
                    for qc in range(2):
                        nc.tensor.matmul(
                            ps[:, qc * 512 : (qc + 1) * 512],
                            qkT[6 + hp][ro : ro + 64, kc * 128 : (kc + 1) * 128],
                            qkT[hp][ro : ro + 64, qc * 512 : (qc + 1) * 512],
                            start=True,
                            stop=True,
                        )
                        off = 1023 - kc * 128 + qc * 512
                        nc.vector.tensor_add(
                            ps[:, qc * 512 : (qc + 1) * 512],
                            ps[:, qc * 512 : (qc + 1) * 512],
                            rb[:, off : off + 512],
                        )
                    pt = ptp.tile([128, 1024], bf16, tag="pt", name="pt")
                    nc.scalar.activation(pt[:], ps[:], AF.Exp)
                    ptiles.append(pt)
                for qc in range(2):
                    pav = psAV.tile([128, 512], f32, tag="pav", name="pav")
                    for kc in range(NT):
                        nc.tensor.matmul(
                            pav[0:65, :],
                            vaug[kc][:, h * 65 : (h + 1) * 65],
                            ptiles[kc][:, qc * 512 : (qc + 1) * 512],
                            start=(kc == 0),
                            stop=(kc == NT - 1),
                        )
                    if odd:
                        tmp = oddp.tile([128, 512], f32, tag="odd", name="avodd")
                        nc.vector.tensor_copy(tmp[0:64, :], pav[0:64, :])
                        nc.sync.dma_start(
                            aT[hp][64:128, qc * 512 : (qc + 1) * 512], tmp[0:64, :]
                        )
                    else:
                        nc.vector.tensor_copy(
                            aT[hp][0:64, qc * 512 : (qc + 1) * 512], pav[0:64, :]
                        )
                    srow = srowp.tile([128, 512], f32, tag="srow", name="srow")
                    nc.vector.tensor_copy(srow[64:65, :], pav[64:65, :])
                    nc.sync.dma_start(
                        s_all[h : h + 1, qc * 512 : (qc + 1) * 512], srow[64:65, :]
                    )

        # normalize attention outputs: aT[hp] *= 1/sums (per head, per query)
        with (
            tc.tile_pool(name="stgp", bufs=4) as stgp,
            tc.tile_pool(name="psNorm", bufs=2, space="PSUM") as psN,
        ):
            nc.vector.reciprocal(s_all[:], s_all[:])
            for hp in range(KT):
                for qc in range(2):
                    st0 = stgp.tile([1, 512], f32, tag="stg", name="st0")
                    nc.sync.dma_start(
                        st0[:], s_all[2 * hp : 2 * hp + 1, qc * 512 : (qc + 1) * 512]
                    )
                    st1 = stgp.tile([1, 512], f32, tag="stg", name="st1")
                    nc.sync.dma_start(
                        st1[:], s_all[2 * hp + 1 : 2 * hp + 2, qc * 512 : (qc + 1) * 512]
                    )
                    psb = psN.tile([128, 512], f32, tag="psn", name="psn")
                    nc.tensor.matmul(
                        psb[0:64, :], ones64[:], st0[:], start=True, stop=True,
                        tile_position=(0, 0),
                    )
                    nc.tensor.matmul(
                        psb[64:128, :], ones64[:], st1[:], start=True, stop=True,
                        tile_position=(0, 64),
                    )
                    nc.vector.tensor_mul(
                        aT[hp][:, qc * 512 : (qc + 1) * 512],
                        aT[hp][:, qc * 512 : (qc + 1) * 512],
                        psb[:],
                    )

        # ---------------- phase E: proj
        yT = [fm_tile(f"yT{i}") for i in range(KT)]
        with tc.tile_pool(name="wpp", bufs=KT) as wpp:
            wp = []
            for c in range(KT):
                wt = wpp.tile([128, C], f32, tag="wp", name=f"wp{c}")
                nc.sync.dma_start(wt[:], wproj_d[c * 128 : (c + 1) * 128, :])
                wp.append(wt)
            with tc.tile_pool(name="psE", bufs=3, space="PSUM") as psE:
                for co in range(KT):
                    for qc in range(2):
                        ps = psE.tile([128, 512], f32, tag="ps", name="pse")
                        for c in range(KT):
                            nc.tensor.matmul(
                                ps[:],
                                wp[c][:, co * 128 : (co + 1) * 128],
                                aT[c][:, qc * 512 : (qc + 1) * 512],
                                start=(c == 0),
                                stop=(c == KT - 1),
                            )
                        nc.scalar.activation(
                            yT[co][:, qc * 512 : (qc + 1) * 512],
                            ps[:],
                            AF.Identity,
                            bias=bproj_sb[:, co : co + 1],
                        )

        # ---------------- phase F: transpose y, residual in-place, LN2, -> h2T
        h2T = [fm_tile(f"h2T{i}") for i in range(KT)]
        with tc.tile_pool(name="psF", bufs=4, space="PSUM") as psF:
            for t in range(NT):
                for ct in range(KT):
                    ps = psF.tile([128, 128], f32, tag="tp", name="psf")
                    nc.tensor.transpose(
                        ps[:], yT[ct][:, t * 128 : (t + 1) * 128], ident[:]
                    )
                    nc.vector.tensor_add(
                        xs[t][:, ct * 128 : (ct + 1) * 128],
                        xs[t][:, ct * 128 : (ct + 1) * 128],
                        ps[:],
                    )
                h2 = chain.tile([128, C], f32, tag="hln", bufs=3, name=f"h2_{t}")
                layernorm(h2[:], xs[t][:], h2[:])
                for ct in range(KT):
                    ps = psF.tile([128, 128], f32, tag="tp", name="psf2")
                    nc.tensor.transpose(
                        ps[:], h2[:, ct * 128 : (ct + 1) * 128], ident[:]
                    )
                    nc.any.tensor_copy(h2T[ct][:, t * 128 : (t + 1) * 128], ps[:])

        # ---------------- phase H: MLP + final residual + store
        with (
            tc.tile_pool(name="w1p", bufs=3) as w1p,
            tc.tile_pool(name="w2p", bufs=3) as w2p,
            tc.tile_pool(name="grp", bufs=3) as grp,
            tc.tile_pool(name="o2p", bufs=6) as o2p,
            tc.tile_pool(name="obp", bufs=2) as obp,
        ):
            for qc in range(2):
                with tc.tile_pool(name="psO", bufs=6, space="PSUM") as ps_o:
                    pso = [
                        ps_o.tile([128, 512], f32, tag="pso", name=f"pso{qc}_{i}")
                        for i in range(KT)
                    ]
                    with tc.tile_pool(name="psG2", bufs=2, space="PSUM") as ps_g:
                        for r in range(RT):
                            w1 = w1p.tile([128, C], f32, tag="w1", name=f"w1_{r}")
                            nc.sync.dma_start(w1[:], wfc1_d[r])
                            w2 = w2p.tile([128, C], f32, tag="w2", name=f"w2_{r}")
                            nc.sync.dma_start(
                                w2[:], wfc2_d[r * 128 : (r + 1) * 128, :]
                            )
                            psg = ps_g.tile([128, 512], f32, tag="psg", name="psg")
                            for ct in range(KT):
                                nc.tensor.matmul(
                                    psg[:],
                                    w1[:, ct * 128 : (ct + 1) * 128],
                                    h2T[ct][:, qc * 512 : (qc + 1) * 512],
                                    start=(ct == 0),
                                    stop=(ct == KT - 1),
                                )
                            gr = grp.tile([128, 512], f32, tag="gr", name="gr")
                            nc.scalar.activation(
                                gr[:], psg[:], AF.Gelu, bias=bfc1_sb[:, r : r + 1]
                            )
                            for co in range(KT):
                                nc.tensor.matmul(
                                    pso[co][:],
                                    w2[:, co * 128 : (co + 1) * 128],
                                    gr[:],
                                    start=(r == 0),
                                    stop=(r == RT - 1),
                                )
                    o2 = []
                    for co in range(KT):
                        o2t = o2p.tile([128, 512], f32, tag="o2", name=f"o2_{qc}_{co}")
                        nc.scalar.activation(
                            o2t[:], pso[co][:], AF.Identity,
                            bias=bfc2_sb[:, co : co + 1],
                        )
                        o2.append(o2t)
                with tc.tile_pool(name="psH", bufs=2, space="PSUM") as psH:
                    for t4 in range(4):
                        t = qc * 4 + t4
                        ob = obp.tile([128, C], f32, tag="ob", name="ob")
                        for co in range(KT):
                            ps = psH.tile([128, 128], f32, tag="tp", name="psh")
                            nc.tensor.transpose(
                                ps[:], o2[co][:, t4 * 128 : (t4 + 1) * 128], ident[:]
                            )
                            nc.vector.tensor_add(
                                ob[:, co * 128 : (co + 1) * 128],
                                xs[t][:, co * 128 : (co + 1) * 128],
                                ps[:],
                            )
                        nc.sync.dma_start(out_d[t * 128 : (t + 1) * 128, :], ob[:])

    nc.compile()
    return nc


def _get_nc():
    if "nc" not in _NC_CACHE:
        _NC_CACHE["nc"] = _build_nc()
    return _NC_CACHE["nc"]


def _host_prep(inputs):
    import ml_dtypes

    inp = {k: np.asarray(v) for k, v in inputs.items()}
    x = np.ascontiguousarray(inp["x"], dtype=np.float32)  # [8, 1024, 768]
    g1 = inp["ln1_g"].astype(np.float64)
    b1 = inp["ln1_b"].astype(np.float64)
    qkv_w = inp["qkv_w"].astype(np.float64)  # [2304, 768]
    Ws = qkv_w.copy()
    Ws[:C] *= D ** (-0.5)  # fold attention scale into Wq
    wqkvT = np.ascontiguousarray((Ws * g1[None, :]).T).astype(np.float32)  # [768, 2304]
    bqkv = (Ws @ b1).astype(np.float32).reshape(1, 3 * C)

    wprojT = np.ascontiguousarray(inp["proj_w"].astype(np.float32).T)  # [768, 768]
    bproj = inp["proj_b"].astype(np.float32).reshape(1, C)

    g2 = inp["ln2_g"].astype(np.float64)
    b2 = inp["ln2_b"].astype(np.float64)
    fc1_w = inp["fc1_w"].astype(np.float64)  # [3072, 768]
    wfc1T = (fc1_w * g2[None, :]).T.astype(np.float32)  # [768, 3072]
    # pre-tiled: wfc1t[r, p, ct*128+j] = wfc1T[ct*128+p, r*128+j]
    wfc1t = np.ascontiguousarray(
        wfc1T.reshape(KT, 128, RT, 128).transpose(2, 1, 0, 3).reshape(RT, 128, C)
    )
    bfc1 = (fc1_w @ b2 + inp["fc1_b"].astype(np.float64)).astype(np.float32)
    bfc1 = bfc1.reshape(1, F1)
    wfc2T = np.ascontiguousarray(inp["fc2_w"].astype(np.float32).T)  # [3072, 768]
    bfc2 = inp["fc2_b"].astype(np.float32).reshape(1, C)

    # rel-bias toeplitz band: rband[h, p, w] = rel_table[clip(p + 1087 - w, 0, 128), h]
    tab = inp["rel_table"].astype(np.float32)  # [129, 12]
    p_i = np.arange(128)
    w_i = np.arange(W)
    idx = np.clip(p_i[:, None] + (N + 63) - w_i[None, :], 0, 2 * 64)
    rband = np.ascontiguousarray(tab[idx, :].transpose(2, 0, 1)).astype(
        ml_dtypes.bfloat16
    )  # [12, 128, 2047]

    shared = {
        "wqkvT": wqkvT,
        "bqkv": bqkv,
        "wprojT": wprojT,
        "bproj": bproj,
        "wfc1t": wfc1t,
        "bfc1": bfc1,
        "wfc2T": wfc2T,
        "bfc2": bfc2,
        "rband": rband,
    }
    in_maps = [{"x": np.ascontiguousarray(x[c]), **shared} for c in range(B)]
    return in_maps


def _make_runner():
    import jax
    from jax.experimental.shard_map import shard_map
    from jax.sharding import Mesh, NamedSharding, PartitionSpec

    from concourse import bass2jax, mybir

    nc = _get_nc()
    bass2jax.install_neuronx_cc_hook()

    partition_name = nc.partition_id_tensor.name if nc.partition_id_tensor else None
    in_names, out_names, out_avals, zero_outs = [], [], [], []
    for alloc in nc.m.functions[0].allocations:
        if not isinstance(alloc, mybir.MemoryLocationSet):
            continue
        name = alloc.memorylocations[0].name
        if alloc.kind == "ExternalInput":
            if name != partition_name:
                in_names.append(name)
        elif alloc.kind == "ExternalOutput":
            out_names.append(name)
            shape = tuple(alloc.tensor_shape)
            dtype = mybir.dt.np(alloc.dtype)
            out_avals.append(jax.core.ShapedArray(shape, dtype))
            zero_outs.append(np.zeros(shape, dtype))
    n_params = len(in_names)
    all_names = tuple(in_names) + tuple(out_names)
    if partition_name is not None:
        all_names = all_names + (partition_name,)
    donate = tuple(range(n_params, n_params + len(out_names)))

    def _body(*args):
        operands = list(args)
        if partition_name is not None:
            operands.append(bass2jax.partition_id_tensor())
        outs = bass2jax._bass_exec_p.bind(
            *operands,
            out_avals=tuple(out_avals),
            in_names=all_names,
            out_names=tuple(out_names),
            lowering_input_output_aliases=(),
            sim_require_finite=True,
            sim_require_nnan=True,
            nc=nc,
        )
        return tuple(outs)

    devices = jax.devices()[:B]
    mesh = Mesh(np.asarray(devices), ("core",))
    in_specs = (PartitionSpec("core"),) * (n_params + len(out_names))
    out_specs = (PartitionSpec("core"),) * len(out_names)
    fn = jax.jit(
        shard_map(
            _body, mesh=mesh, in_specs=in_specs, out_specs=out_specs, check_rep=False
        ),
        donate_argnums=donate,
        keep_unused=True,
    )
    sharding = NamedSharding(mesh, PartitionSpec("core"))
    return fn, in_names, out_names, zero_outs, sharding


def _get_runner():
    if "runner" not in _NC_CACHE:
        _NC_CACHE["runner"] = _make_runner()
    return _NC_CACHE["runner"]


LAST_BENCH = None


def kernel(**inputs):
    global LAST_BENCH
    import time

    import jax

    fn, in_names, out_names, zero_outs, sharding = _get_runner()
    in_maps = _host_prep(inputs)
    concat_in = [
        np.concatenate([np.asarray(in_maps[c][n]) for c in range(B)], axis=0)
        for n in in_names
    ]
    concat_zeros = [
        np.zeros((B * z.shape[0], *z.shape[1:]), z.dtype) for z in zero_outs
    ]
    dev_in = [jax.device_put(a, sharding) for a in concat_in]
    outs = fn(*dev_in, *concat_zeros)
    jax.block_until_ready(outs)
    result = np.asarray(outs[0]).reshape(B, N, C).astype(np.float32)

    iters = int(os.environ.get("BENCH_ITERS", "0"))
    if iters > 0:
        o = fn(*dev_in, *outs)  # warm
        jax.block_until_ready(o)
        t0 = time.perf_counter()
        for _ in range(iters):
            o = fn(*dev_in, *o)
        jax.block_until_ready(o)
        dt = (time.perf_counter() - t0) / iters
        LAST_BENCH = {"per_iter_ns": dt * 1e9, "iters": iters}
    return result
